# revision 2
# baseline (speedup 1.0000x reference)
"""RNN-T Joiner kernel for Trainium2, data-parallel over (B, T) on 8 cores.

reference:
    logit = tanh(enc[:, :, None, :] + dec[:, None, :, :])   # (B,T,U,C)
    out   = einsum('btuc,vc->btuv', logit, W) + b           # (B,T,U,V)

Shapes (hardcoded): B=4, T=256, U=64, C=512, V=1024.

Sharding: core k handles b = k//2, t rows [ (k%2)*128, (k%2)*128+128 ).
W / bias replicated. No collectives.

v2: bf16 matmul datapath (W and logits in bf16; bf16 moving operand
streams ~2x faster than fp32r), bf16 output stores (half the HBM store
traffic; host converts back to f32), c-major matmul order so each logit
weight-load serves both vh psum tiles, merged [128,1024] stores
alternating across the two HWDGE queues (sync/scalar).
"""

import numpy as np

B, T, U, C, V = 4, 256, 64, 512, 1024
NCORES = 8
TS = 128  # t rows per core
CCH = C // 128  # 4 contraction chunks
VH = V // 512  # 2 psum-width chunks

_CACHE = {}


def _build(repeat=1):
    from contextlib import ExitStack

    import concourse.bacc as bacc
    import concourse.mybir as mybir
    import concourse.tile as tile

    dt = mybir.dt
    f32 = dt.float32
    bf16 = dt.bfloat16

    nc = bacc.Bacc("TRN2", target_bir_lowering=False, debug=False, num_devices=NCORES)
    enc_t = nc.declare_dram_parameter("enc_t", [C, TS], f32, isOutput=False)
    dec_t = nc.declare_dram_parameter("dec_t", [C, U], f32, isOutput=False)
    wt = nc.declare_dram_parameter("wt", [C, V], bf16, isOutput=False)
    bias_rep = nc.declare_dram_parameter("bias_rep", [128, V], f32, isOutput=False)
    out = nc.declare_dram_parameter("out", [TS, U, V], bf16, isOutput=True)

    with tile.TileContext(nc) as tc, ExitStack() as ctx:
        const = ctx.enter_context(tc.tile_pool(name="const", bufs=1))
        logit_pool = ctx.enter_context(tc.tile_pool(name="logit", bufs=8))
        psum_pool = ctx.enter_context(tc.tile_pool(name="psum", bufs=6, space="PSUM"))
        warm_pool = ctx.enter_context(tc.tile_pool(name="warm", bufs=1, space="PSUM"))
        out_pool = ctx.enter_context(tc.tile_pool(name="out", bufs=8))

        wt_sb = const.tile([128, CCH * V], bf16, tag="wt")
        enc_sb = const.tile([128, CCH * TS], f32, tag="enc")
        dec_sb = const.tile([128, CCH * U], f32, tag="dec")
        bias_sb = const.tile([128, V], f32, tag="bias")
        scratch = const.tile([128, 1], f32, tag="scratch")

        # Preload the tanh activation table while input DMAs run.
        nc.vector.memset(scratch[:], 0.0)
        nc.scalar.activation(
            scratch[:], scratch[:], mybir.ActivationFunctionType.Tanh
        )
        # Warm the PE clock (p-state ramps with continuous work) during the
        # input-DMA window with throwaway matmuls on a spare PSUM bank.
        warm_sb = const.tile([128, 512], bf16, tag="warm_sb")
        warm = warm_pool.tile([128, 512], f32, tag="warm")
        nc.vector.memset(warm_sb[:].bitcast(f32), 0.0)
        for _ in range(8):
            nc.tensor.matmul(
                warm[:],
                lhsT=warm_sb[:, 0:128],
                rhs=warm_sb[:],
                start=True,
                stop=True,
            )

        nc.gpsimd.dma_start(enc_sb[:, 0:TS], enc_t[0:128, :])
        nc.gpsimd.dma_start(
            dec_sb[:].rearrange("p (c u) -> p c u", c=CCH),
            dec_t[:].rearrange("(c p) u -> p c u", p=128),
        )
        nc.gpsimd.dma_start(
            enc_sb[:, TS:].rearrange("p (c t) -> p c t", c=CCH - 1),
            enc_t[128:, :].rearrange("(c p) t -> p c t", p=128),
        )
        nc.sync.dma_start(wt_sb[:, 0:V], wt[0:128, :])
        nc.scalar.dma_start(wt_sb[:, V : 2 * V], wt[128:256, :])
        nc.sync.dma_start(wt_sb[:, 2 * V : 3 * V], wt[256:384, :])
        nc.scalar.dma_start(wt_sb[:, 3 * V : 4 * V], wt[384:512, :])
        nc.gpsimd.dma_start(bias_sb[:], bias_rep[:])

        store_q = [nc.sync, nc.scalar]
        for i, u in enumerate([u for _ in range(repeat) for u in range(U)]):
            lg = logit_pool.tile([128, CCH * TS], bf16, tag="lg")
            for c in range(CCH):
                nc.scalar.activation(
                    lg[:, c * TS : (c + 1) * TS],
                    enc_sb[:, c * TS : (c + 1) * TS],
                    mybir.ActivationFunctionType.Tanh,
                    bias=dec_sb[:, c * U + u : c * U + u + 1],
                )
            ps = [psum_pool.tile([128, 512], f32, tag=f"ps{vh}") for vh in range(VH)]
            for c in range(CCH):
                for vh in range(VH):
                    nc.tensor.matmul(
                        ps[vh][:],
                        lhsT=lg[:, c * TS : (c + 1) * TS],
                        rhs=wt_sb[:, c * V + vh * 512 : c * V + vh * 512 + 512],
                        start=(c == 0),
                        stop=(c == CCH - 1),
                    )
            ob = out_pool.tile([128, V], bf16, tag="ob")
            for vh in range(VH):
                nc.vector.tensor_add(
                    ob[:, vh * 512 : (vh + 1) * 512],
                    ps[vh][:],
                    bias_sb[:, vh * 512 : (vh + 1) * 512],
                )
            store_q[i % 2].dma_start(out[:, u, :], ob[:])

    nc.finalize()
    return nc


def _get_nc():
    if "nc" not in _CACHE:
        _CACHE["nc"] = _build()
    return _CACHE["nc"]


def kernel(**inputs):
    import ml_dtypes

    enc = np.asarray(inputs["enc_out"], dtype=np.float32)
    dec = np.asarray(inputs["dec_out"], dtype=np.float32)
    W = np.asarray(inputs["W"], dtype=np.float32)
    b = np.asarray(inputs["b"], dtype=np.float32)

    nc = _get_nc()

    wt_np = np.ascontiguousarray(W.T).astype(ml_dtypes.bfloat16)
    bias_np = np.ascontiguousarray(np.broadcast_to(b, (128, V)))
    in_maps = []
    for k in range(NCORES):
        bb, t0 = k // 2, (k % 2) * TS
        in_maps.append(
            {
                "enc_t": np.ascontiguousarray(enc[bb, t0 : t0 + TS, :].T),
                "dec_t": np.ascontiguousarray(dec[bb].T),
                "wt": wt_np,
                "bias_rep": bias_np,
            }
        )

    from concourse.bass_utils import run_bass_kernel_spmd

    res = run_bass_kernel_spmd(nc, in_maps, list(range(NCORES)))
    _CACHE["last_result"] = res

    out = np.empty((B, T, U, V), np.float32)
    for k in range(NCORES):
        bb, t0 = k // 2, (k % 2) * TS
        out[bb, t0 : t0 + TS] = res.results[k]["out"].astype(np.float32)
    return out


# revision 4
# speedup vs baseline: 1.1407x; 1.1407x over previous
"""RNN-T Joiner kernel for Trainium2, data-parallel over (B, T) on 8 cores.

reference:
    logit = tanh(enc[:, :, None, :] + dec[:, None, :, :])   # (B,T,U,C)
    out   = einsum('btuc,vc->btuv', logit, W) + b           # (B,T,U,V)

Shapes (hardcoded): B=4, T=256, U=64, C=512, V=1024.

Sharding: core k handles b = k//2, t rows [ (k%2)*128, (k%2)*128+128 ).
W / bias replicated. No collectives.

v2: bf16 matmul datapath (W and logits in bf16; bf16 moving operand
streams ~2x faster than fp32r), bf16 output stores (half the HBM store
traffic; host converts back to f32), c-major matmul order so each logit
weight-load serves both vh psum tiles, merged [128,1024] stores
alternating across the two HWDGE queues (sync/scalar).
"""

import numpy as np

B, T, U, C, V = 4, 256, 64, 512, 1024
NCORES = 8
TS = 128  # t rows per core
CCH = C // 128  # 4 contraction chunks
VH = V // 512  # 2 psum-width chunks

_CACHE = {}


def _build(repeat=1):
    from contextlib import ExitStack

    import concourse.bacc as bacc
    import concourse.mybir as mybir
    import concourse.tile as tile

    dt = mybir.dt
    f32 = dt.float32
    bf16 = dt.bfloat16

    nc = bacc.Bacc("TRN2", target_bir_lowering=False, debug=False, num_devices=NCORES)
    enc_t = nc.declare_dram_parameter("enc_t", [C, TS], f32, isOutput=False)
    dec_t = nc.declare_dram_parameter("dec_t", [C, U], f32, isOutput=False)
    wt = nc.declare_dram_parameter("wt", [C, V], bf16, isOutput=False)
    bias_rep = nc.declare_dram_parameter("bias_rep", [128, V], f32, isOutput=False)
    out = nc.declare_dram_parameter("out", [TS, U, V], bf16, isOutput=True)

    with tile.TileContext(nc) as tc, ExitStack() as ctx:
        const = ctx.enter_context(tc.tile_pool(name="const", bufs=1))
        logit_pool = ctx.enter_context(tc.tile_pool(name="logit", bufs=8))
        psum_pool = ctx.enter_context(tc.tile_pool(name="psum", bufs=3, space="PSUM"))
        warm_pool = ctx.enter_context(tc.tile_pool(name="warm", bufs=1, space="PSUM"))
        out_pool = ctx.enter_context(tc.tile_pool(name="out", bufs=8))

        wt_sb = const.tile([128, CCH * V], bf16, tag="wt")
        enc_sb = const.tile([128, CCH * TS], f32, tag="enc")
        dec_sb = const.tile([128, CCH * U], f32, tag="dec")
        bias_sb = const.tile([128, V], f32, tag="bias")
        scratch = const.tile([128, 1], f32, tag="scratch")

        # Preload the tanh activation table while input DMAs run.
        nc.vector.memset(scratch[:], 0.0)
        nc.scalar.activation(
            scratch[:], scratch[:], mybir.ActivationFunctionType.Tanh
        )
        # Warm the PE clock (p-state ramps with continuous work) during the
        # input-DMA window with throwaway matmuls on a spare PSUM bank.
        warm_sb = const.tile([128, 512], bf16, tag="warm_sb")
        warm = warm_pool.tile([128, 512], f32, tag="warm")
        nc.vector.memset(warm_sb[:].bitcast(f32), 0.0)
        for _ in range(8):
            nc.tensor.matmul(
                warm[:],
                lhsT=warm_sb[:, 0:128],
                rhs=warm_sb[:],
                start=True,
                stop=True,
            )

        nc.gpsimd.dma_start(enc_sb[:, 0:TS], enc_t[0:128, :])
        nc.gpsimd.dma_start(
            dec_sb[:].rearrange("p (c u) -> p c u", c=CCH),
            dec_t[:].rearrange("(c p) u -> p c u", p=128),
        )
        nc.gpsimd.dma_start(
            enc_sb[:, TS:].rearrange("p (c t) -> p c t", c=CCH - 1),
            enc_t[128:, :].rearrange("(c p) t -> p c t", p=128),
        )
        nc.sync.dma_start(wt_sb[:, 0:V], wt[0:128, :])
        nc.scalar.dma_start(wt_sb[:, V : 2 * V], wt[128:256, :])
        nc.sync.dma_start(wt_sb[:, 2 * V : 3 * V], wt[256:384, :])
        nc.scalar.dma_start(wt_sb[:, 3 * V : 4 * V], wt[384:512, :])
        nc.gpsimd.dma_start(bias_sb[:], bias_rep[:])

        store_q = [nc.sync, nc.scalar]
        for i, u in enumerate([u for _ in range(repeat) for u in range(U)]):
            lg = logit_pool.tile([128, CCH * TS], bf16, tag="lg")
            for c in range(CCH):
                nc.scalar.activation(
                    lg[:, c * TS : (c + 1) * TS],
                    enc_sb[:, c * TS : (c + 1) * TS],
                    mybir.ActivationFunctionType.Tanh,
                    bias=dec_sb[:, c * U + u : c * U + u + 1],
                )
            ps0 = psum_pool.tile([128, 512], f32, tag="ps0")
            ps1 = psum_pool.tile([128, 512], f32, tag="ps1")
            ps = [ps0, ps1]
            for c in range(CCH):
                for vh in range(VH):
                    nc.tensor.matmul(
                        ps[vh][:],
                        lhsT=lg[:, c * TS : (c + 1) * TS],
                        rhs=wt_sb[:, c * V + vh * 512 : c * V + vh * 512 + 512],
                        start=(c == 0),
                        stop=(c == CCH - 1),
                    )
            ob = out_pool.tile([128, V], bf16, tag="ob")
            for vh in range(VH):
                nc.vector.tensor_add(
                    ob[:, vh * 512 : (vh + 1) * 512],
                    ps[vh][:],
                    bias_sb[:, vh * 512 : (vh + 1) * 512],
                )
            store_q[i % 2].dma_start(out[:, u, :], ob[:])

    nc.finalize()
    return nc


def _get_nc():
    if "nc" not in _CACHE:
        _CACHE["nc"] = _build()
    return _CACHE["nc"]


def kernel(**inputs):
    import ml_dtypes

    enc = np.asarray(inputs["enc_out"], dtype=np.float32)
    dec = np.asarray(inputs["dec_out"], dtype=np.float32)
    W = np.asarray(inputs["W"], dtype=np.float32)
    b = np.asarray(inputs["b"], dtype=np.float32)

    nc = _get_nc()

    wt_np = np.ascontiguousarray(W.T).astype(ml_dtypes.bfloat16)
    bias_np = np.ascontiguousarray(np.broadcast_to(b, (128, V)))
    in_maps = []
    for k in range(NCORES):
        bb, t0 = k // 2, (k % 2) * TS
        in_maps.append(
            {
                "enc_t": np.ascontiguousarray(enc[bb, t0 : t0 + TS, :].T),
                "dec_t": np.ascontiguousarray(dec[bb].T),
                "wt": wt_np,
                "bias_rep": bias_np,
            }
        )

    from concourse.bass_utils import run_bass_kernel_spmd

    res = run_bass_kernel_spmd(nc, in_maps, list(range(NCORES)))
    _CACHE["last_result"] = res

    out = np.empty((B, T, U, V), np.float32)
    for k in range(NCORES):
        bb, t0 = k // 2, (k % 2) * TS
        out[bb, t0 : t0 + TS] = res.results[k]["out"].astype(np.float32)
    return out
